# revision 3
# baseline (speedup 1.0000x reference)
"""AngleLoss distributed Trainium2 kernel (v2).

mean(arccos(dot(o,t) / (|o||t|))) over 2,097,152 rows of 3-vectors,
data-parallel over 8 NeuronCores (no collective: each core returns
per-partition partial sums; host adds 16*128 floats per core).

Math per row (division- and sign-free, bf16 compute):
    dot = sum o*t ; oo = sum o^2 ; tt = sum t^2
    c    = dot * absrsqrt(oo*tt)              # cos(theta)
    nump = min(c-1, 0)                        # = -relu(1-c)
    r2   = absrsqrt(|1 - c^2|)
    g    = nump * r2                          # = -tan(theta/2)
    theta = -2*arctan(g), accumulated per chunk via accum_out.

v2 layout/pipeline vs v1:
  - inputs stored bf16 in DRAM (host converts during sharding): halves
    HBM traffic; DVE 2x perf mode applies to every tensor_tensor.
  - per front tile [P, 6, F] planar; products written into a 9-plane
    buffer [m0 m1 m2 | ox2 oy2 oz2 | tx2 ty2 tz2] so ONE strided pair-add
    (planes {0,3,6}+{1,4,7}) then one more (+{2,5,8}) yields
    [dot|oo|tt] planar - 2 instructions for all three reductions.
  - engine balance: Act does 4/6 squares + c^2 + the table ops
    (absrsqrt x2, arctan); Pool (gpsimd) does the first pair-add for all
    but the last tile; VE does the rest. Big instructions (few-hundred ns
    each) amortize the ~130ns/instr overhead that dominated v1.
"""

import os as _os
import sys

import numpy as np

if "/opt/trn_rl_repo" not in sys.path:
    sys.path.insert(0, "/opt/trn_rl_repo")

N_CORES = 8
R_TOTAL = 256 * 8192  # 2097152 rows
PER_CORE = R_TOTAL // N_CORES  # 262144
P = 128
FREE = PER_CORE // P  # 2048


def _env_tuple(name, default):
    v = _os.environ.get(name)
    return tuple(int(x) for x in v.split(",")) if v else default


# front tiles (DMA + products + reductions); taper at the end to shorten
# the drain chain
FRONT = _env_tuple("ANGLE_FRONT", (512, 512, 384, 384, 256))
# tail chunks (prod/c/c2/nump/r2/g/atan); chunk j starts once the fronts
# covering it have produced b
CHUNK = _env_tuple("ANGLE_CHUNK", (1024, 640, 384))
# how many of the 6 square-planes VE computes (rest on Act); planes are
# ordered [ox oy oz tx ty tz]
SQ_ON_VE = int(_os.environ.get("ANGLE_SQ_ON_VE", "2"))
# pool computes pair-add "a" for tiles 0..T-2 (last tile's on VE to keep
# the critical tail short); set 0 to disable pool entirely
POOL_A = int(_os.environ.get("ANGLE_POOL_A", "1"))
assert sum(FRONT) == FREE and sum(CHUNK) == FREE

_BUILD_CACHE = {}


def _build_nc():
    key = (FRONT, CHUNK, SQ_ON_VE, POOL_A)
    if key in _BUILD_CACHE:
        return _BUILD_CACHE[key]

    from concourse import bacc, mybir

    AF = mybir.ActivationFunctionType
    OP = mybir.AluOpType
    f32 = mybir.dt.float32
    bf16 = mybir.dt.bfloat16

    T = len(FRONT)
    C = len(CHUNK)
    Fmax = max(FRONT)
    ofs = [0]
    for s in FRONT:
        ofs.append(ofs[-1] + s)
    cfs = [0]
    for s in CHUNK:
        cfs.append(cfs[-1] + s)
    # chunk j needs this many b-tiles done
    need_b = []
    for j in range(C):
        n = 0
        while ofs[n] < cfs[j + 1]:
            n += 1
        need_b.append(n)
    # cumulative dma-queue totals (2 queues, 16 per completion)
    NQ = 2
    tot = {}
    slot_tot = [0] * NQ
    for i in range(T):
        slot_tot[i % NQ] += 16
        tot[i] = slot_tot[i % NQ]

    nc = bacc.Bacc(
        "TRN2", target_bir_lowering=False, debug=False, num_devices=N_CORES
    )
    x = nc.dram_tensor("x", [6 * P * FREE], bf16, kind="ExternalInput")
    out = nc.dram_tensor("out", [P, 16], f32, kind="ExternalOutput")
    xf = x.ap()

    def sb(name, shape, dtype):
        return nc.alloc_sbuf_tensor(name, list(shape), dtype).ap()

    inb = [sb(f"inb{b}", [P, 6 * Fmax], bf16) for b in range(2)]
    pl = [sb(f"pl{b}", [P, 9 * Fmax], bf16) for b in range(2)]
    ab = [sb(f"ab{b}", [P, 3 * Fmax], bf16) for b in range(2)]
    B = sb("B", [P, 3 * FREE], bf16)
    prodb = sb("prodb", [P, FREE], bf16)
    r1b = sb("r1b", [P, FREE], bf16)
    cb = sb("cb", [P, FREE], bf16)
    c2v = sb("c2v", [P, FREE], bf16)
    numpb = sb("numpb", [P, FREE], bf16)
    r2b = sb("r2b", [P, FREE], bf16)
    gall = sb("gall", [P, FREE], bf16)
    tscr = sb("tscr", [P, FREE], bf16)
    asum = sb("asum", [P, 16], f32)
    warm = sb("warm", [P, 1], bf16)
    bias0 = sb("bias0", [P, 1], f32)
    bias1 = sb("bias1", [P, 1], f32)

    S_dq = [nc.alloc_semaphore(f"s_dq{q}") for q in range(NQ)]
    S_bias = nc.alloc_semaphore("s_bias")
    S_vf = nc.alloc_semaphore("s_vf")  # 1/tile: VE front (m+sq)
    S_af = nc.alloc_semaphore("s_af")  # 1/tile: Act front (sq4)
    S_pa = nc.alloc_semaphore("s_pa")  # 1/tile: pair-add a
    S_b = nc.alloc_semaphore("s_b")  # 1/tile: b written
    S_pr = nc.alloc_semaphore("s_pr")  # 1/chunk: prod
    S_c = nc.alloc_semaphore("s_c")  # 1/chunk: c
    S_r1 = nc.alloc_semaphore("s_r1")  # 1/chunk
    S_r2 = nc.alloc_semaphore("s_r2")  # 1/chunk
    S_g = nc.alloc_semaphore("s_g")  # 1/chunk
    S_fin = nc.alloc_semaphore("s_fin")
    S_dmo = nc.alloc_semaphore("s_dmo")

    B3 = B.rearrange("p (j f) -> p j f", j=3)  # planes dot|oo|tt

    def pl9(bidx, F):
        return pl[bidx][:, : 9 * F].rearrange("p (j f) -> p j f", j=9)

    def a3(bidx, F):
        return ab[bidx][:, : 3 * F].rearrange("p (j f) -> p j f", j=3)

    nsq_a = 6 - SQ_ON_VE  # planes on Act, from the top

    with nc.Block(no_gpsimd_drain=True) as block:

        @block.sync
        def _(sync):
            for i in range(T):
                if i >= 2:
                    # inbuf reuse: tile i-2 fully consumed by both fronts
                    sync.wait_ge(S_vf, i - 1)
                    sync.wait_ge(S_af, i - 1)
                tile = xf[6 * P * ofs[i] : 6 * P * ofs[i + 1]].rearrange(
                    "(p f) -> p f", p=P
                )
                sync.dma_start(
                    out=inb[i % 2][:, : 6 * FRONT[i]], in_=tile
                ).then_inc(S_dq[i % NQ], 16)
            sync.wait_ge(S_fin, 1)
            sync.dma_start(out=out.ap()[:, :], in_=asum[:, :]).then_inc(
                S_dmo, 16
            )
            sync.wait_ge(S_dmo, 16)

        if POOL_A:

            @block.gpsimd
            def _(gpsimd):
                for i in range(T - 1):
                    F = FRONT[i]
                    gpsimd.wait_ge(S_vf, i + 1)
                    gpsimd.wait_ge(S_af, i + 1)
                    if i >= 2:
                        # ab[i%2] free once b of tile i-2 has read it
                        gpsimd.wait_ge(S_b, i - 1)
                    p9 = pl9(i % 2, F)
                    gpsimd.tensor_tensor(
                        a3(i % 2, F)[:], p9[:, 0:7:3, :], p9[:, 1:8:3, :],
                        OP.add,
                    ).then_inc(S_pa)

        @block.vector
        def _(vector):
            vector.memset(bias0[:], 0.0).then_inc(S_bias)
            vector.memset(bias1[:], 1.0).then_inc(S_bias)
            vector.memset(asum[:, :], 0.0).then_inc(S_bias)

            issued_b = 0
            next_prod = 0
            next_c = 0
            next_g = 0

            def front(i):
                F = FRONT[i]
                vector.wait_ge(S_dq[i % NQ], tot[i])
                vector.tensor_tensor(
                    pl[i % 2][:, : 3 * F],
                    inb[i % 2][:, : 3 * F],
                    inb[i % 2][:, 3 * F : 6 * F],
                    OP.mult,
                )
                vector.tensor_tensor(
                    pl[i % 2][:, 3 * F : (3 + SQ_ON_VE) * F],
                    inb[i % 2][:, : SQ_ON_VE * F],
                    inb[i % 2][:, : SQ_ON_VE * F],
                    OP.mult,
                ).then_inc(S_vf)

            def bstage(i):
                nonlocal issued_b
                F = FRONT[i]
                p9 = pl9(i % 2, F)
                if i == T - 1 or not POOL_A:
                    # last tile's pair-add on VE (pool would land too late)
                    vector.wait_ge(S_af, i + 1)
                    vector.tensor_tensor(
                        a3(i % 2, F)[:], p9[:, 0:7:3, :], p9[:, 1:8:3, :],
                        OP.add,
                    ).then_inc(S_pa)
                vector.wait_ge(S_pa, i + 1)
                vector.tensor_tensor(
                    B3[:, :, ofs[i] : ofs[i + 1]],
                    a3(i % 2, F)[:],
                    p9[:, 2:9:3, :],
                    OP.add,
                ).then_inc(S_b)
                issued_b += 1

            def try_prod():
                nonlocal next_prod
                while next_prod < C and need_b[next_prod] <= issued_b:
                    j = next_prod
                    sl = slice(cfs[j], cfs[j + 1])
                    vector.tensor_tensor(
                        prodb[:, sl], B3[:, 1, sl], B3[:, 2, sl], OP.mult
                    ).then_inc(S_pr)
                    next_prod += 1

            def cstage(j):
                sl = slice(cfs[j], cfs[j + 1])
                vector.wait_ge(S_r1, j + 1)
                vector.tensor_tensor(
                    cb[:, sl], B3[:, 0, sl], r1b[:, sl], OP.mult
                ).then_inc(S_c)
                vector.tensor_scalar(
                    numpb[:, sl], cb[:, sl], 1.0, 0.0, OP.subtract, OP.min
                )

            def gstage(j):
                sl = slice(cfs[j], cfs[j + 1])
                vector.wait_ge(S_r2, j + 1)
                vector.tensor_tensor(
                    gall[:, sl], numpb[:, sl], r2b[:, sl], OP.mult
                ).then_inc(S_g)

            # schedule: fronts and b interleaved; chunk stages slotted in
            # as their inputs appear, lagging so cross-engine round trips
            # (r1, r2 on Act) are never same-slot
            for k in range(T):
                front(k)
                if k >= 1:
                    bstage(k - 1)
                    try_prod()
                # c-stage for chunks whose prod went out >= 2 b-slots ago
                while next_c < next_prod - 1:
                    cstage(next_c)
                    next_c += 1
                while next_g < next_c - 1:
                    gstage(next_g)
                    next_g += 1
            bstage(T - 1)
            try_prod()
            while next_c < C:
                cstage(next_c)
                next_c += 1
                if next_g < next_c - 1:
                    gstage(next_g)
                    next_g += 1
            while next_g < C:
                gstage(next_g)
                next_g += 1

        @block.scalar
        def _(scalar):
            # first activation in program order pins the absrsqrt table set
            scalar.activation(
                warm[:], warm[:], AF.Abs_reciprocal_sqrt, bias=warm[:],
                scale=0.0,
            )
            scalar.wait_ge(S_bias, 3)

            next_r1 = 0
            next_r2 = 0

            def sq4(i):
                F = FRONT[i]
                scalar.wait_ge(S_dq[i % NQ], tot[i])
                if i >= 2:
                    # pl[i%2] planes 5..8 free once b of tile i-2 read them
                    scalar.wait_ge(S_b, i - 1)
                scalar.activation(
                    pl[i % 2][:, (9 - nsq_a) * F : 9 * F],
                    inb[i % 2][:, (6 - nsq_a) * F : 6 * F],
                    AF.Square,
                    bias=bias0[:],
                ).then_inc(S_af)

            def r1stage(j):
                sl = slice(cfs[j], cfs[j + 1])
                scalar.wait_ge(S_pr, j + 1)
                scalar.activation(
                    r1b[:, sl], prodb[:, sl], AF.Abs_reciprocal_sqrt,
                    bias=bias0[:],
                ).then_inc(S_r1)

            def r2stage(j):
                sl = slice(cfs[j], cfs[j + 1])
                scalar.wait_ge(S_c, j + 1)
                scalar.activation(
                    c2v[:, sl], cb[:, sl], AF.Square, bias=bias0[:]
                )
                scalar.activation(
                    r2b[:, sl], c2v[:, sl], AF.Abs_reciprocal_sqrt,
                    bias=bias1[:], scale=-1.0,
                ).then_inc(S_r2)

            for i in range(T):
                sq4(i)
                # keep r1 as early as possible so VE's c never starves
                while next_r1 < C and need_b[next_r1] <= i:
                    r1stage(next_r1)
                    next_r1 += 1
                while next_r2 < next_r1 - 1:
                    r2stage(next_r2)
                    next_r2 += 1
            while next_r1 < C:
                r1stage(next_r1)
                next_r1 += 1
            while next_r2 < C:
                r2stage(next_r2)
                next_r2 += 1
            # dummy arctan: loads the sigmoid-set tables while VE finishes g
            scalar.activation(
                warm[:], warm[:], AF.Arctan, bias=bias0[:], scale=0.0
            )
            for j in range(C):
                sl = slice(cfs[j], cfs[j + 1])
                scalar.wait_ge(S_g, j + 1)
                scalar.activation(
                    tscr[:, sl], gall[:, sl], AF.Arctan, bias=bias0[:],
                    accum_out=asum[:, j : j + 1],
                )
            # the accumulator drains via a separate READ_ACCUMULATOR uop
            # after ACTIVATE; a trailing in-order op must carry the final
            # semaphore so the out-DMA cannot read asum early
            scalar.activation(
                warm[:], warm[:], AF.Copy, bias=0.0, scale=0.0
            ).then_inc(S_fin)

    nc.compile()
    _BUILD_CACHE[key] = nc
    return nc


def _shard_inputs(outputs, targets):
    import ml_dtypes

    bf = ml_dtypes.bfloat16
    o = np.asarray(outputs, dtype=np.float32).reshape(-1, 3)
    t = np.asarray(targets, dtype=np.float32).reshape(-1, 3)
    in_maps = []
    for cidx in range(N_CORES):
        lo, hi = cidx * PER_CORE, (cidx + 1) * PER_CORE
        planes = np.empty((6, P, FREE), dtype=bf)
        for k in range(3):
            planes[k] = o[lo:hi, k].astype(bf).reshape(P, FREE)
            planes[3 + k] = t[lo:hi, k].astype(bf).reshape(P, FREE)
        blocks = []
        off = 0
        for F in FRONT:
            blk = planes[:, :, off : off + F]  # [6, P, F]
            blocks.append(
                np.ascontiguousarray(blk.transpose(1, 0, 2)).reshape(-1)
            )
            off += F
        in_maps.append({"x": np.concatenate(blocks)})
    return in_maps


LAST_RESULT = None


def kernel(outputs, targets):
    global LAST_RESULT
    import os

    from concourse.bass_utils import run_bass_kernel_spmd

    nc = _build_nc()
    in_maps = _shard_inputs(outputs, targets)
    trace = bool(os.environ.get("ANGLE_KERNEL_TRACE"))
    res = run_bass_kernel_spmd(
        nc, in_maps, core_ids=list(range(N_CORES)), trace=trace
    )
    LAST_RESULT = res
    total = 0.0
    for rmap in res.results:
        total += np.asarray(rmap["out"], dtype=np.float64).sum()
    # device accumulates sum(arctan(-g)); theta = -2*arctan(g)
    mean = -2.0 * total / R_TOTAL
    return np.float32(mean)


# revision 4
# speedup vs baseline: 1.1998x; 1.1998x over previous
"""AngleLoss distributed Trainium2 kernel (v3).

mean(arccos(dot(o,t) / (|o||t|))) over 2,097,152 rows of 3-vectors,
data-parallel over 8 NeuronCores (no collective: each core returns
per-partition partial sums; host adds 16*128 floats per core).

Math per row (division- and sign-free, bf16 compute):
    dot = sum o*t ; oo = sum o^2 ; tt = sum t^2
    c    = dot * absrsqrt(oo*tt)              # cos(theta)
    nump = min(c-1, 0)                        # = -relu(1-c)
    r2   = absrsqrt(|1 - c^2|)
    g    = nump * r2                          # = -tan(theta/2)
    theta = -2*arctan(g), accumulated via accum_out.

Findings baked in (from HW traces):
  - bf16 inputs (host converts during shard): halves HBM traffic and all
    tensor_tensor ops hit the DVE 2x perf mode (0.52ns/el/lane measured);
    tensor_scalar hits 4x.
  - gpsimd tensor_tensor CONTENDS with DVE (~4x DVE slowdown while a pool
    op streams) -> pool does no compute here, it only issues input DMAs
    (its sequencer boots ~1.3us before sync's, so tile0 lands earlier).
  - teardown scales with semaphore count (~45-115ns per sem reset per
    engine, serialized) -> 6 semaphores total.
  - per front tile [P, 6, F] planar; products into a 9-plane buffer
    [m0 m1 m2 | ox2 oy2 oz2 | tx2 ty2 tz2]; two strided pair-adds
    (planes {0,3,6}+{1,4,7}, then +{2,5,8}) give [dot|oo|tt] planar.
  - single arctan at the end (one table switch, one accumulator drain);
    tapered fronts and chunks keep the drain chain short.
"""

import os as _os
import sys

import numpy as np

if "/opt/trn_rl_repo" not in sys.path:
    sys.path.insert(0, "/opt/trn_rl_repo")

N_CORES = 8
R_TOTAL = 256 * 8192  # 2097152 rows
PER_CORE = R_TOTAL // N_CORES  # 262144
P = 128
FREE = PER_CORE // P  # 2048


def _env_tuple(name, default):
    v = _os.environ.get(name)
    return tuple(int(x) for x in v.split(",")) if v else default


FRONT = _env_tuple("ANGLE_FRONT", (192, 512, 512, 512, 320))
CHUNK = _env_tuple("ANGLE_CHUNK", (896, 768, 384))
# square planes computed on VE (0..5); the remaining 6-SQ_ON_VE on Act
SQ_ON_VE = int(_os.environ.get("ANGLE_SQ_ON_VE", "2"))
# which engine issues input DMAs: gpsimd boots earliest
DMA_ENG = _os.environ.get("ANGLE_DMA_ENG", "gpsimd")
assert sum(FRONT) == FREE and sum(CHUNK) == FREE
assert 1 <= SQ_ON_VE <= 5

_BUILD_CACHE = {}


def _build_nc():
    key = (FRONT, CHUNK, SQ_ON_VE, DMA_ENG)
    if key in _BUILD_CACHE:
        return _BUILD_CACHE[key]

    from concourse import bacc, mybir

    AF = mybir.ActivationFunctionType
    OP = mybir.AluOpType
    f32 = mybir.dt.float32
    bf16 = mybir.dt.bfloat16

    T = len(FRONT)
    C = len(CHUNK)
    Fmax = max(FRONT)
    ofs = [0]
    for s in FRONT:
        ofs.append(ofs[-1] + s)
    cfs = [0]
    for s in CHUNK:
        cfs.append(cfs[-1] + s)
    need_b = []  # chunk j needs this many b-tiles
    for j in range(C):
        n = 0
        while ofs[n] < cfs[j + 1]:
            n += 1
        need_b.append(n)

    nc = bacc.Bacc(
        "TRN2", target_bir_lowering=False, debug=False, num_devices=N_CORES
    )
    x = nc.dram_tensor("x", [6 * P * FREE], bf16, kind="ExternalInput")
    out = nc.dram_tensor("out", [P, 16], f32, kind="ExternalOutput")
    xf = x.ap()

    def sb(name, shape, dtype):
        return nc.alloc_sbuf_tensor(name, list(shape), dtype).ap()

    inb = [sb(f"inb{b}", [P, 6 * Fmax], bf16) for b in range(2)]
    pl = [sb(f"pl{b}", [P, 9 * Fmax], bf16) for b in range(2)]
    ab = [sb(f"ab{b}", [P, 3 * Fmax], bf16) for b in range(2)]
    B = sb("B", [P, 3 * FREE], bf16)
    prodb = sb("prodb", [P, FREE], bf16)
    r1b = sb("r1b", [P, FREE], bf16)
    cb = sb("cb", [P, FREE], bf16)
    c2v = sb("c2v", [P, FREE], bf16)
    numpb = sb("numpb", [P, FREE], bf16)
    r2b = sb("r2b", [P, FREE], bf16)
    gall = sb("gall", [P, FREE], bf16)
    tscr = sb("tscr", [P, FREE], bf16)
    asum = sb("asum", [P, 16], f32)
    warm = sb("warm", [P, 1], bf16)
    bias0 = sb("bias0", [P, 1], f32)
    bias1 = sb("bias1", [P, 1], f32)

    S_dq = nc.alloc_semaphore("s_dq")  # dma completions, +16 each
    S_cons = nc.alloc_semaphore("s_cons")  # memset(+1), VE/Act fronts (+1 ea)
    S_vt = nc.alloc_semaphore("s_vt")  # VE progress: b/prod/c-group/g incs
    S_at = nc.alloc_semaphore("s_at")  # Act progress: r1/r2 incs
    S_fin = nc.alloc_semaphore("s_fin")
    S_dmo = nc.alloc_semaphore("s_dmo")

    B3 = B.rearrange("p (j f) -> p j f", j=3)  # planes dot|oo|tt

    def pl9(bidx, F):
        return pl[bidx][:, : 9 * F].rearrange("p (j f) -> p j f", j=9)

    def a3(bidx, F):
        return ab[bidx][:, : 3 * F].rearrange("p (j f) -> p j f", j=3)

    nsq_a = 6 - SQ_ON_VE

    # static positions of incs on the cross-engine progress sems
    vt_pos = {}  # name -> value after inc
    at_pos = {}
    vt_n = 0
    at_n = 0

    def vt_inc(name):
        nonlocal vt_n
        vt_n += 1
        vt_pos[name] = vt_n

    def at_inc(name):
        nonlocal at_n
        at_n += 1
        at_pos[name] = at_n

    # --- plan VE order (names) ---
    ve_order = []
    issued_b = 0
    np_, nc_, ng_ = 0, 0, 0

    def plan_tail():
        nonlocal np_, nc_, ng_
        while np_ < C and need_b[np_] <= issued_b:
            ve_order.append(("prod", np_))
            np_ += 1
        while nc_ < np_ - 1:
            ve_order.append(("cgrp", nc_))
            nc_ += 1
        while ng_ < nc_ - 1:
            ve_order.append(("g", ng_))
            ng_ += 1

    for k in range(T):
        ve_order.append(("front", k))
        if k >= 1:
            ve_order.append(("b", k - 1))
            issued_b += 1
            plan_tail()
    ve_order.append(("b", T - 1))
    issued_b += 1
    plan_tail()
    while nc_ < C:
        ve_order.append(("cgrp", nc_))
        nc_ += 1
        while ng_ < nc_ - 1:
            ve_order.append(("g", ng_))
            ng_ += 1
    while ng_ < C:
        ve_order.append(("g", ng_))
        ng_ += 1
    # record vt positions in this order
    for st, idx in ve_order:
        if st == "b":
            vt_inc(f"b{idx}")
        elif st == "prod":
            vt_inc(f"prod{idx}")
        elif st == "cgrp":
            vt_inc(f"c2_{idx}")
        elif st == "g":
            vt_inc(f"g{idx}")

    # --- plan Act order ---
    act_order = []
    nr1, nr2 = 0, 0
    for i in range(T):
        act_order.append(("sq", i))
        ib = i - 1  # b-tiles guaranteed issued by VE before our wait
        while nr1 < C and need_b[nr1] <= ib:
            act_order.append(("r1", nr1))
            nr1 += 1
        while nr2 < nr1 - 1:
            act_order.append(("r2", nr2))
            nr2 += 1
    while nr1 < C:
        act_order.append(("r1", nr1))
        nr1 += 1
    while nr2 < C:
        act_order.append(("r2", nr2))
        nr2 += 1
    for st, idx in act_order:
        if st == "r1":
            at_inc(f"r1_{idx}")
        elif st == "r2":
            at_inc(f"r2_{idx}")

    with nc.Block(no_gpsimd_drain=True) as block:

        def emit_in_dmas(eng):
            for i in range(T):
                if i >= 2:
                    # inbuf reuse: tile i-2 consumed by both fronts
                    eng.wait_ge(S_cons, 1 + 2 * (i - 1))
                tile = xf[6 * P * ofs[i] : 6 * P * ofs[i + 1]].rearrange(
                    "(p f) -> p f", p=P
                )
                eng.dma_start(
                    out=inb[i % 2][:, : 6 * FRONT[i]], in_=tile
                ).then_inc(S_dq, 16)

        @block.sync
        def _(sync):
            if DMA_ENG == "sync":
                emit_in_dmas(sync)
            sync.wait_ge(S_fin, 1)
            sync.dma_start(out=out.ap()[:, :], in_=asum[:, :]).then_inc(
                S_dmo, 16
            )
            sync.wait_ge(S_dmo, 16)

        if DMA_ENG == "gpsimd":

            @block.gpsimd
            def _(gpsimd):
                emit_in_dmas(gpsimd)

        @block.vector
        def _(vector):
            vector.memset(bias0[:], 0.0)
            vector.memset(bias1[:], 1.0)
            vector.memset(asum[:, :], 0.0).then_inc(S_cons)

            def front(i):
                F = FRONT[i]
                vector.wait_ge(S_dq, 16 * (i + 1))
                vector.tensor_tensor(
                    pl[i % 2][:, : 3 * F],
                    inb[i % 2][:, : 3 * F],
                    inb[i % 2][:, 3 * F : 6 * F],
                    OP.mult,
                )
                vector.tensor_tensor(
                    pl[i % 2][:, 3 * F : (3 + SQ_ON_VE) * F],
                    inb[i % 2][:, : SQ_ON_VE * F],
                    inb[i % 2][:, : SQ_ON_VE * F],
                    OP.mult,
                ).then_inc(S_cons)

            def bstage(i):
                F = FRONT[i]
                p9 = pl9(i % 2, F)
                # a+b read Act's square planes of tile i
                vector.wait_ge(S_cons, 2 * i + 3)
                vector.tensor_tensor(
                    a3(i % 2, F)[:], p9[:, 0:7:3, :], p9[:, 1:8:3, :], OP.add
                )
                vector.tensor_tensor(
                    B3[:, :, ofs[i] : ofs[i + 1]],
                    a3(i % 2, F)[:],
                    p9[:, 2:9:3, :],
                    OP.add,
                ).then_inc(S_vt)

            def prod(j):
                sl = slice(cfs[j], cfs[j + 1])
                vector.tensor_tensor(
                    prodb[:, sl], B3[:, 1, sl], B3[:, 2, sl], OP.mult
                ).then_inc(S_vt)

            def cgrp(j):
                sl = slice(cfs[j], cfs[j + 1])
                vector.wait_ge(S_at, at_pos[f"r1_{j}"])
                vector.tensor_tensor(
                    cb[:, sl], B3[:, 0, sl], r1b[:, sl], OP.mult
                )
                vector.tensor_scalar(
                    numpb[:, sl], cb[:, sl], 1.0, 0.0, OP.subtract, OP.min
                )
                vector.tensor_tensor(
                    c2v[:, sl], cb[:, sl], cb[:, sl], OP.mult
                ).then_inc(S_vt)

            def gstage(j):
                sl = slice(cfs[j], cfs[j + 1])
                vector.wait_ge(S_at, at_pos[f"r2_{j}"])
                vector.tensor_tensor(
                    gall[:, sl], numpb[:, sl], r2b[:, sl], OP.mult
                ).then_inc(S_vt)

            fns = {"front": front, "b": bstage, "prod": prod, "cgrp": cgrp,
                   "g": gstage}
            for st, idx in ve_order:
                fns[st](idx)

        @block.scalar
        def _(scalar):
            # first activation in program order pins the absrsqrt table set
            scalar.activation(
                warm[:], warm[:], AF.Abs_reciprocal_sqrt, bias=warm[:],
                scale=0.0,
            )
            scalar.wait_ge(S_cons, 1)

            def sq(i):
                F = FRONT[i]
                scalar.wait_ge(S_dq, 16 * (i + 1))
                if i >= 2:
                    # pl[i%2] square planes free once b of tile i-2 read them
                    scalar.wait_ge(S_vt, vt_pos[f"b{i - 2}"])
                scalar.activation(
                    pl[i % 2][:, (9 - nsq_a) * F : 9 * F],
                    inb[i % 2][:, (6 - nsq_a) * F : 6 * F],
                    AF.Square,
                    bias=bias0[:],
                ).then_inc(S_cons)

            def r1(j):
                sl = slice(cfs[j], cfs[j + 1])
                scalar.wait_ge(S_vt, vt_pos[f"prod{j}"])
                scalar.activation(
                    r1b[:, sl], prodb[:, sl], AF.Abs_reciprocal_sqrt,
                    bias=bias0[:],
                ).then_inc(S_at)

            def r2(j):
                sl = slice(cfs[j], cfs[j + 1])
                scalar.wait_ge(S_vt, vt_pos[f"c2_{j}"])
                scalar.activation(
                    r2b[:, sl], c2v[:, sl], AF.Abs_reciprocal_sqrt,
                    bias=bias1[:], scale=-1.0,
                ).then_inc(S_at)

            fns = {"sq": sq, "r1": r1, "r2": r2}
            for st, idx in act_order:
                fns[st](idx)

            # dummy arctan: loads the sigmoid-set tables while VE finishes g
            scalar.activation(
                warm[:], warm[:], AF.Arctan, bias=bias0[:], scale=0.0
            )
            scalar.wait_ge(S_vt, vt_pos[f"g{C - 1}"])
            scalar.activation(
                tscr[:, :], gall[:, :], AF.Arctan, bias=bias0[:],
                accum_out=asum[:, 0:1],
            )
            # accumulator drains via a separate uop after ACTIVATE; trailing
            # op carries the final semaphore so the out-DMA can't read early
            scalar.activation(
                warm[:], warm[:], AF.Copy, bias=0.0, scale=0.0
            ).then_inc(S_fin)

    nc.compile()
    _BUILD_CACHE[key] = nc
    return nc


def _shard_inputs(outputs, targets):
    import ml_dtypes

    bf = ml_dtypes.bfloat16
    o = np.asarray(outputs, dtype=np.float32).reshape(-1, 3)
    t = np.asarray(targets, dtype=np.float32).reshape(-1, 3)
    in_maps = []
    for cidx in range(N_CORES):
        lo, hi = cidx * PER_CORE, (cidx + 1) * PER_CORE
        planes = np.empty((6, P, FREE), dtype=bf)
        for k in range(3):
            planes[k] = o[lo:hi, k].astype(bf).reshape(P, FREE)
            planes[3 + k] = t[lo:hi, k].astype(bf).reshape(P, FREE)
        blocks = []
        off = 0
        for F in FRONT:
            blk = planes[:, :, off : off + F]  # [6, P, F]
            blocks.append(
                np.ascontiguousarray(blk.transpose(1, 0, 2)).reshape(-1)
            )
            off += F
        in_maps.append({"x": np.concatenate(blocks)})
    return in_maps


LAST_RESULT = None


def kernel(outputs, targets):
    global LAST_RESULT
    import os

    from concourse.bass_utils import run_bass_kernel_spmd

    nc = _build_nc()
    in_maps = _shard_inputs(outputs, targets)
    trace = bool(os.environ.get("ANGLE_KERNEL_TRACE"))
    res = run_bass_kernel_spmd(
        nc, in_maps, core_ids=list(range(N_CORES)), trace=trace
    )
    LAST_RESULT = res
    total = 0.0
    for rmap in res.results:
        total += np.asarray(rmap["out"], dtype=np.float64).sum()
    # device accumulates sum(arctan(-g)); theta = -2*arctan(g)
    mean = -2.0 * total / R_TOTAL
    return np.float32(mean)


# revision 9
# speedup vs baseline: 1.2143x; 1.0121x over previous
"""AngleLoss distributed Trainium2 kernel (v3).

mean(arccos(dot(o,t) / (|o||t|))) over 2,097,152 rows of 3-vectors,
data-parallel over 8 NeuronCores (no collective: each core returns
per-partition partial sums; host adds 16*128 floats per core).

Math per row (division- and sign-free, bf16 compute):
    dot = sum o*t ; oo = sum o^2 ; tt = sum t^2
    c    = dot * absrsqrt(oo*tt)              # cos(theta)
    nump = min(c-1, 0)                        # = -relu(1-c)
    r2   = absrsqrt(|1 - c^2|)
    g    = nump * r2                          # = -tan(theta/2)
    theta = -2*arctan(g), accumulated via accum_out.

Findings baked in (from HW traces):
  - bf16 inputs (host converts during shard): halves HBM traffic and all
    tensor_tensor ops hit the DVE 2x perf mode (0.52ns/el/lane measured);
    tensor_scalar hits 4x.
  - gpsimd tensor_tensor CONTENDS with DVE (~4x DVE slowdown while a pool
    op streams) -> pool does no compute here, it only issues input DMAs
    (its sequencer boots ~1.3us before sync's, so tile0 lands earlier).
  - teardown scales with semaphore count (~45-115ns per sem reset per
    engine, serialized) -> 6 semaphores total.
  - per front tile [P, 6, F] planar; products into a 9-plane buffer
    [m0 m1 m2 | ox2 oy2 oz2 | tx2 ty2 tz2]; two strided pair-adds
    (planes {0,3,6}+{1,4,7}, then +{2,5,8}) give [dot|oo|tt] planar.
  - single arctan at the end (one table switch, one accumulator drain);
    tapered fronts and chunks keep the drain chain short.
"""

import os as _os
import sys

import numpy as np

if "/opt/trn_rl_repo" not in sys.path:
    sys.path.insert(0, "/opt/trn_rl_repo")

N_CORES = 8
R_TOTAL = 256 * 8192  # 2097152 rows
PER_CORE = R_TOTAL // N_CORES  # 262144
P = 128
FREE = PER_CORE // P  # 2048


def _env_tuple(name, default):
    v = _os.environ.get(name)
    return tuple(int(x) for x in v.split(",")) if v else default


FRONT = _env_tuple("ANGLE_FRONT", (192, 512, 512, 512, 320))
CHUNK = _env_tuple("ANGLE_CHUNK", (896, 768, 384))
# square planes computed on VE (0..5); the remaining 6-SQ_ON_VE on Act
SQ_ON_VE = int(_os.environ.get("ANGLE_SQ_ON_VE", "1"))
# which engine issues input DMAs (gpsimd-issued DMA measured 7us slower)
DMA_ENG = _os.environ.get("ANGLE_DMA_ENG", "sync")
# skip the out-DMA completion wait: block teardown's dma_reset drains it
SKIP_DMO_WAIT = int(_os.environ.get("ANGLE_SKIP_DMO", "0"))
assert sum(FRONT) == FREE and sum(CHUNK) == FREE
assert 1 <= SQ_ON_VE <= 5 and len(CHUNK) >= 2

_BUILD_CACHE = {}


def _build_nc():
    key = (FRONT, CHUNK, SQ_ON_VE, DMA_ENG, SKIP_DMO_WAIT)
    if key in _BUILD_CACHE:
        return _BUILD_CACHE[key]

    from concourse import bacc, mybir

    AF = mybir.ActivationFunctionType
    OP = mybir.AluOpType
    f32 = mybir.dt.float32
    bf16 = mybir.dt.bfloat16

    T = len(FRONT)
    C = len(CHUNK)
    Fmax = max(FRONT)
    ofs = [0]
    for s in FRONT:
        ofs.append(ofs[-1] + s)
    cfs = [0]
    for s in CHUNK:
        cfs.append(cfs[-1] + s)
    need_b = []  # chunk j needs this many b-tiles
    for j in range(C):
        n = 0
        while ofs[n] < cfs[j + 1]:
            n += 1
        need_b.append(n)

    nc = bacc.Bacc(
        "TRN2", target_bir_lowering=False, debug=False, num_devices=N_CORES
    )
    x = nc.dram_tensor("x", [6 * P * FREE], bf16, kind="ExternalInput")
    out = nc.dram_tensor("out", [P, 16], f32, kind="ExternalOutput")
    xf = x.ap()

    def sb(name, shape, dtype):
        return nc.alloc_sbuf_tensor(name, list(shape), dtype).ap()

    inb = [sb(f"inb{b}", [P, 6 * Fmax], bf16) for b in range(2)]
    pl = [sb(f"pl{b}", [P, 9 * Fmax], bf16) for b in range(2)]
    ab = [sb(f"ab{b}", [P, 3 * Fmax], bf16) for b in range(2)]
    B = sb("B", [P, 3 * FREE], bf16)
    prodb = sb("prodb", [P, FREE], bf16)
    r1b = sb("r1b", [P, FREE], bf16)
    cb = sb("cb", [P, FREE], bf16)
    c2v = sb("c2v", [P, FREE], bf16)
    numpb = sb("numpb", [P, FREE], bf16)
    r2b = sb("r2b", [P, FREE], bf16)
    gall = sb("gall", [P, FREE], bf16)
    tscr = sb("tscr", [P, FREE], bf16)
    asum = sb("asum", [P, 16], f32)
    warm = sb("warm", [P, 1], bf16)
    bias0 = sb("bias0", [P, 1], f32)
    bias1 = sb("bias1", [P, 1], f32)

    S_dq = nc.alloc_semaphore("s_dq")  # dma completions, +16 each
    S_cons = nc.alloc_semaphore("s_cons")  # memset(+1), VE/Act fronts (+1 ea)
    S_vt = nc.alloc_semaphore("s_vt")  # VE progress: b/prod/c-group/g incs
    S_at = nc.alloc_semaphore("s_at")  # Act progress: r1/r2 incs
    S_fin = nc.alloc_semaphore("s_fin")
    S_dmo = nc.alloc_semaphore("s_dmo")

    B3 = B.rearrange("p (j f) -> p j f", j=3)  # planes dot|oo|tt

    def pl9(bidx, F):
        return pl[bidx][:, : 9 * F].rearrange("p (j f) -> p j f", j=9)

    def a3(bidx, F):
        return ab[bidx][:, : 3 * F].rearrange("p (j f) -> p j f", j=3)

    nsq_a = 6 - SQ_ON_VE

    # static positions of incs on the cross-engine progress sems
    vt_pos = {}  # name -> value after inc
    at_pos = {}
    vt_n = 0
    at_n = 0

    def vt_inc(name):
        nonlocal vt_n
        vt_n += 1
        vt_pos[name] = vt_n

    def at_inc(name):
        nonlocal at_n
        at_n += 1
        at_pos[name] = at_n

    # --- plan VE order (names) ---
    ve_order = []
    issued_b = 0
    np_, nc_, ng_ = 0, 0, 0

    def plan_tail():
        # prods ASAP; cgrps lag one prod (covers the r1 round trip on Act);
        # all g-ops last so the arctan table load overlaps them
        nonlocal np_, nc_, ng_
        while np_ < C and need_b[np_] <= issued_b:
            ve_order.append(("prod", np_))
            np_ += 1
        while nc_ < np_ - 1:
            ve_order.append(("cgrp", nc_))
            nc_ += 1

    for k in range(T):
        ve_order.append(("front", k))
        if k >= 1:
            ve_order.append(("b", k - 1))
            issued_b += 1
            plan_tail()
    ve_order.append(("b", T - 1))
    issued_b += 1
    plan_tail()
    while nc_ < C:
        ve_order.append(("cgrp", nc_))
        nc_ += 1
    while ng_ < C:
        ve_order.append(("g", ng_))
        ng_ += 1
    # record vt positions in this order
    for st, idx in ve_order:
        if st == "b":
            vt_inc(f"b{idx}")
        elif st == "prod":
            vt_inc(f"prod{idx}")
        elif st == "cgrp":
            vt_inc(f"c2_{idx}")
        elif st == "g":
            vt_inc(f"g{idx}")

    # --- plan Act order ---
    act_order = []
    nr1, nr2 = 0, 0
    for i in range(T):
        act_order.append(("sq", i))
        ib = i - 1  # b-tiles guaranteed issued by VE before our wait
        while nr1 < C and need_b[nr1] <= ib:
            act_order.append(("r1", nr1))
            nr1 += 1
        while nr2 < nr1 - 1:
            act_order.append(("r2", nr2))
            nr2 += 1
    while nr1 < C:
        act_order.append(("r1", nr1))
        nr1 += 1
    while nr2 < C:
        act_order.append(("r2", nr2))
        nr2 += 1
    for st, idx in act_order:
        if st == "r1":
            at_inc(f"r1_{idx}")
        elif st == "r2":
            at_inc(f"r2_{idx}")

    with nc.Block(no_gpsimd_drain=True) as block:

        def emit_in_dmas(eng):
            for i in range(T):
                if i >= 2:
                    # inbuf reuse: tile i-2 consumed by both fronts
                    eng.wait_ge(S_cons, 1 + 2 * (i - 1))
                tile = xf[6 * P * ofs[i] : 6 * P * ofs[i + 1]].rearrange(
                    "(p f) -> p f", p=P
                )
                eng.dma_start(
                    out=inb[i % 2][:, : 6 * FRONT[i]], in_=tile
                ).then_inc(S_dq, 16)

        @block.sync
        def _(sync):
            if DMA_ENG == "sync":
                emit_in_dmas(sync)
            sync.wait_ge(S_fin, 1)
            sync.dma_start(out=out.ap()[:, :], in_=asum[:, :]).then_inc(
                S_dmo, 16
            )
            if not SKIP_DMO_WAIT:
                sync.wait_ge(S_dmo, 16)

        if DMA_ENG == "gpsimd":

            @block.gpsimd
            def _(gpsimd):
                emit_in_dmas(gpsimd)

        @block.vector
        def _(vector):
            vector.memset(bias0[:], 0.0)
            vector.memset(bias1[:], 1.0)
            vector.memset(asum[:, :], 0.0).then_inc(S_cons)

            def front(i):
                F = FRONT[i]
                vector.wait_ge(S_dq, 16 * (i + 1))
                vector.tensor_tensor(
                    pl[i % 2][:, : 3 * F],
                    inb[i % 2][:, : 3 * F],
                    inb[i % 2][:, 3 * F : 6 * F],
                    OP.mult,
                )
                vector.tensor_tensor(
                    pl[i % 2][:, 3 * F : (3 + SQ_ON_VE) * F],
                    inb[i % 2][:, : SQ_ON_VE * F],
                    inb[i % 2][:, : SQ_ON_VE * F],
                    OP.mult,
                ).then_inc(S_cons)

            def bstage(i):
                F = FRONT[i]
                p9 = pl9(i % 2, F)
                # a+b read Act's square planes of tile i
                vector.wait_ge(S_cons, 2 * i + 3)
                vector.tensor_tensor(
                    a3(i % 2, F)[:], p9[:, 0:7:3, :], p9[:, 1:8:3, :], OP.add
                )
                vector.tensor_tensor(
                    B3[:, :, ofs[i] : ofs[i + 1]],
                    a3(i % 2, F)[:],
                    p9[:, 2:9:3, :],
                    OP.add,
                ).then_inc(S_vt)

            def prod(j):
                sl = slice(cfs[j], cfs[j + 1])
                vector.tensor_tensor(
                    prodb[:, sl], B3[:, 1, sl], B3[:, 2, sl], OP.mult
                ).then_inc(S_vt)

            def cgrp(j):
                sl = slice(cfs[j], cfs[j + 1])
                vector.wait_ge(S_at, at_pos[f"r1_{j}"])
                vector.tensor_tensor(
                    cb[:, sl], B3[:, 0, sl], r1b[:, sl], OP.mult
                )
                vector.tensor_scalar(
                    numpb[:, sl], cb[:, sl], 1.0, 0.0, OP.subtract, OP.min
                )
                vector.tensor_tensor(
                    c2v[:, sl], cb[:, sl], cb[:, sl], OP.mult
                ).then_inc(S_vt)

            def gstage(j):
                sl = slice(cfs[j], cfs[j + 1])
                vector.wait_ge(S_at, at_pos[f"r2_{j}"])
                vector.tensor_tensor(
                    gall[:, sl], numpb[:, sl], r2b[:, sl], OP.mult
                ).then_inc(S_vt)

            fns = {"front": front, "b": bstage, "prod": prod, "cgrp": cgrp,
                   "g": gstage}
            for st, idx in ve_order:
                fns[st](idx)

        @block.scalar
        def _(scalar):
            # first activation in program order pins the absrsqrt table set
            scalar.activation(
                warm[:], warm[:], AF.Abs_reciprocal_sqrt, bias=warm[:],
                scale=0.0,
            )
            scalar.wait_ge(S_cons, 1)

            def sq(i):
                F = FRONT[i]
                scalar.wait_ge(S_dq, 16 * (i + 1))
                if i >= 2:
                    # pl[i%2] square planes free once b of tile i-2 read them
                    scalar.wait_ge(S_vt, vt_pos[f"b{i - 2}"])
                scalar.activation(
                    pl[i % 2][:, (9 - nsq_a) * F : 9 * F],
                    inb[i % 2][:, (6 - nsq_a) * F : 6 * F],
                    AF.Square,
                    bias=bias0[:],
                ).then_inc(S_cons)

            def r1(j):
                sl = slice(cfs[j], cfs[j + 1])
                scalar.wait_ge(S_vt, vt_pos[f"prod{j}"])
                scalar.activation(
                    r1b[:, sl], prodb[:, sl], AF.Abs_reciprocal_sqrt,
                    bias=bias0[:],
                ).then_inc(S_at)

            def r2(j):
                sl = slice(cfs[j], cfs[j + 1])
                scalar.wait_ge(S_vt, vt_pos[f"c2_{j}"])
                scalar.activation(
                    r2b[:, sl], c2v[:, sl], AF.Abs_reciprocal_sqrt,
                    bias=bias1[:], scale=-1.0,
                ).then_inc(S_at)

            fns = {"sq": sq, "r1": r1, "r2": r2}
            for st, idx in act_order:
                fns[st](idx)

            # dummy arctan: loads the sigmoid-set tables while VE finishes g
            scalar.activation(
                warm[:], warm[:], AF.Arctan, bias=bias0[:], scale=0.0
            )
            # split arctan: the early span only needs g of chunks < C-1,
            # overlapping VE's last g; the final span follows VE's last op
            split = cfs[C - 1]
            scalar.wait_ge(S_vt, vt_pos[f"g{C - 2}"])
            scalar.activation(
                tscr[:, :split], gall[:, :split], AF.Arctan, bias=bias0[:],
                accum_out=asum[:, 0:1],
            )
            scalar.wait_ge(S_vt, vt_pos[f"g{C - 1}"])
            scalar.activation(
                tscr[:, split:], gall[:, split:], AF.Arctan, bias=bias0[:],
                accum_out=asum[:, 1:2],
            )
            # accumulator drains via a separate uop after ACTIVATE; trailing
            # op carries the final semaphore so the out-DMA can't read early
            scalar.activation(
                warm[:], warm[:], AF.Copy, bias=0.0, scale=0.0
            ).then_inc(S_fin)

    nc.compile()
    _BUILD_CACHE[key] = nc
    return nc


def _shard_inputs(outputs, targets):
    import ml_dtypes

    bf = ml_dtypes.bfloat16
    o = np.asarray(outputs, dtype=np.float32).reshape(-1, 3)
    t = np.asarray(targets, dtype=np.float32).reshape(-1, 3)
    in_maps = []
    for cidx in range(N_CORES):
        lo, hi = cidx * PER_CORE, (cidx + 1) * PER_CORE
        planes = np.empty((6, P, FREE), dtype=bf)
        for k in range(3):
            planes[k] = o[lo:hi, k].astype(bf).reshape(P, FREE)
            planes[3 + k] = t[lo:hi, k].astype(bf).reshape(P, FREE)
        blocks = []
        off = 0
        for F in FRONT:
            blk = planes[:, :, off : off + F]  # [6, P, F]
            blocks.append(
                np.ascontiguousarray(blk.transpose(1, 0, 2)).reshape(-1)
            )
            off += F
        in_maps.append({"x": np.concatenate(blocks)})
    return in_maps


LAST_RESULT = None


def kernel(outputs, targets):
    global LAST_RESULT
    import os

    from concourse.bass_utils import run_bass_kernel_spmd

    nc = _build_nc()
    in_maps = _shard_inputs(outputs, targets)
    trace = bool(os.environ.get("ANGLE_KERNEL_TRACE"))
    res = run_bass_kernel_spmd(
        nc, in_maps, core_ids=list(range(N_CORES)), trace=trace
    )
    LAST_RESULT = res
    total = 0.0
    for rmap in res.results:
        total += np.asarray(rmap["out"], dtype=np.float64).sum()
    # device accumulates sum(arctan(-g)); theta = -2*arctan(g)
    mean = -2.0 * total / R_TOTAL
    return np.float32(mean)
